# revision 15
# baseline (speedup 1.0000x reference)
"""DirectedGATLayer Trainium2 Bass kernel (V2).

Math: out[b,j,h,:] = sum_i alpha[b,i,j,h] * Wh[b,i,h,:],
alpha = softmax_i( mask(adj) . LeakyReLU_0.2(s_src[b,i,h] + s_dst[b,j,h]) )

Key identity: exp(leaky(e)) = chi*exp(e) + (1-chi)*exp(0.2e) with
chi = [e >= 0], and exp(e) = exp(s_i)exp(s_j) is rank-1.  Only the
binary chi is materialized at NxN size (one DVE tensor-scalar compare
per tile); all exponentials live on N-sized vectors, and the two leaky
branches plus softmax denominators are accumulated by TensorE matmuls:

  PT[c,j]   = sum_i A[i,c] * chi[i,j],   A = [C1*Whaug | C2*Whaug]
  num/den   = D1_j*PT1 + D2_j*(S2 - PT2)

The -S2 correction rides as an extra row of the (otherwise constant)
identity operand of the per-j-chunk transpose matmul, paired with an
extra ones-row in pt16 — no seed matmuls needed.

s_dst row broadcast to all 128 partitions goes through a DRAM round
trip (SBUF -> DRAM -> stride-0 broadcast read): the DMA engines are
otherwise idle, and a single-partition SBUF source caps at the
per-partition port rate.

adj enters only through exact-zero entries (uniform[0,1) inputs); those
(~1 per run) are corrected exactly on the host from the returned den.

Data parallel over 8 cores: 4 batches per core, identical NEFF (SPMD).
"""
import sys

if '/opt/trn_rl_repo' not in sys.path:
    sys.path.insert(0, '/opt/trn_rl_repo')

import numpy as np
from contextlib import ExitStack

import concourse.bass as bass
import concourse.tile as tile
from concourse import bacc, mybir
from concourse import bass_utils

FP32 = mybir.dt.float32
FP16 = mybir.dt.float16
ALU = mybir.AluOpType
ACTF = mybir.ActivationFunctionType

B, N, D, H, Dh = 32, 512, 128, 4, 32
NB = 4            # batches per core
NCORES = 8
NCH = N // 128    # 4 chunks of 128 along n
AW = 33           # A-block width per head per branch (32 Wh cols + scaled-ones col)
OW = D + H        # out+den column block per j-chunk (132)
ONES_SCALE = 1.0 / 64.0   # keep fp16 denominators in range
NEG_SLOPE = 0.2
MASK_EPS = 1e-8

_CACHE = {}
last_results = None   # BassKernelResults of the most recent run (for test.py)


def _build_nc():
    nc = bacc.Bacc("TRN2", target_bir_lowering=False, debug=False,
                   num_devices=NCORES)

    hx_d = nc.dram_tensor("hx", [NB, N, D], FP16, kind="ExternalInput").ap()
    wt_d = nc.dram_tensor("wt", [D, D + 2 * H], FP16, kind="ExternalInput").ap()
    eye_d = nc.dram_tensor("eye", [D, D], FP16, kind="ExternalInput").ap()
    idext_d = nc.dram_tensor("idext", [2 * AW, H * 2 * AW], FP16,
                             kind="ExternalInput").ap()
    srt_d = nc.dram_tensor("srt", [NB, H, N], FP16, kind="Internal").ap()
    outden_d = nc.dram_tensor("outden", [NB, N, OW], FP32,
                              kind="ExternalOutput").ap()

    with tile.TileContext(nc) as tc, ExitStack() as ctx:
        cpool = ctx.enter_context(tc.tile_pool(name="const", bufs=1))
        sbp = ctx.enter_context(tc.tile_pool(name="sb", bufs=2))
        whp = ctx.enter_context(tc.tile_pool(name="wha", bufs=6))
        ap_pool = ctx.enter_context(tc.tile_pool(name="atile", bufs=8))
        chp = ctx.enter_context(tc.tile_pool(name="chi", bufs=6))
        ptp = ctx.enter_context(tc.tile_pool(name="pt16", bufs=5))
        sjp = ctx.enter_context(tc.tile_pool(name="sjb", bufs=3))
        smp = ctx.enter_context(tc.tile_pool(name="small", bufs=3))
        tlp = ctx.enter_context(tc.tile_pool(name="tail", bufs=2))
        odp = ctx.enter_context(tc.tile_pool(name="od", bufs=2))

        ps_h = ctx.enter_context(tc.tile_pool(name="psH", bufs=1, space="PSUM"))
        ps_wh = ctx.enter_context(tc.tile_pool(name="psW", bufs=2, space="PSUM"))
        ps_pt = ctx.enter_context(tc.tile_pool(name="psPT", bufs=2, space="PSUM"))
        ps_t = ctx.enter_context(tc.tile_pool(name="psT", bufs=3, space="PSUM"))

        # constants
        wt_sb = cpool.tile([D, D + 2 * H], FP16, tag="wt")
        nc.gpsimd.dma_start(wt_sb[:], wt_d[:])
        eye_sb = cpool.tile([D, D], FP16, tag="eye")
        nc.gpsimd.dma_start(eye_sb[:], eye_d[:])
        ones_col = cpool.tile([D, 1], FP16, tag="ones")
        nc.gpsimd.memset(ones_col[:], 1.0)
        lnos = cpool.tile([128, 1], FP32, tag="lnos")
        nc.gpsimd.memset(lnos[:], float(np.log(ONES_SCALE)))
        # two manually double-buffered [identity | -S2-row] tiles
        iextx = [cpool.tile([2 * AW + 1, H * 2 * AW], FP16, tag=f"iextx{k}",
                            name=f"iextx{k}")
                 for k in range(2)]
        for k in range(2):
            nc.gpsimd.dma_start(iextx[k][0:2 * AW, :], idext_d[:])
        actwarm = cpool.tile([1, 1], FP32, tag="actwarm")
        nc.scalar.activation(actwarm[:], ones_col[0:1, 0:1], ACTF.Exp)

        for b in range(NB):
            # ---- Phase A: load h (rows 4p+c on partition p), transpose, Wh, s
            hsb = sbp.tile([128, N], FP16, tag="hsb")
            nc.sync.dma_start(
                hsb[:].rearrange("p (c d) -> p c d", c=NCH),
                hx_d[b].rearrange("(p c) d -> p c d", p=128))

            hT_ps = ps_h.tile([128, N], FP16, tag="hT")
            for ic in range(NCH):
                nc.tensor.transpose(hT_ps[:, ic * 128:(ic + 1) * 128],
                                    hsb[:, ic * 128:(ic + 1) * 128], eye_sb[:])
            hT = sbp.tile([128, N], FP16, tag="hT16")
            nc.scalar.copy(hT[:], hT_ps[:])

            # per chunk: Wh (+ s in the tail cols), whaug, A-tile alloc
            s_sb = smp.tile([128, NCH * 2 * H], FP16, tag="s_sb")
            a_tiles = []
            for cn in range(NCH):
                wh_ps = ps_wh.tile([128, D + 2 * H], FP32, tag="wh")
                nc.tensor.matmul(wh_ps[:], hT[:, cn * 128:(cn + 1) * 128],
                                 wt_sb[:], start=True, stop=True)
                whaug = whp.tile([128, H * AW], FP16, tag="whaug")
                nc.gpsimd.memset(whaug[:], 1.0)
                nc.scalar.copy(
                    whaug[:].rearrange("p (h c) -> p h c", h=H)[:, :, 0:Dh],
                    wh_ps[:, 0:D].rearrange("p (h c) -> p h c", h=H))
                nc.vector.tensor_scalar(
                    s_sb[:, cn * 8:(cn + 1) * 8], wh_ps[:, D:D + 2 * H],
                    1.0, None, ALU.mult)
                at = ap_pool.tile([128, H * 2 * AW], FP16, tag="A")
                a_tiles.append((at, whaug))

            # derived vectors: ssrcn = -s_src (Pool), cc/dall = exp branches
            ssrcn = smp.tile([128, NCH * H], FP32, tag="ssrcn")
            nc.vector.tensor_scalar(
                ssrcn[:].rearrange("p (c h) -> p c h", c=NCH),
                s_sb[:].rearrange("p (c k) -> p c k", c=NCH)[:, :, 0:H],
                -1.0, None, ALU.mult)
            cc = smp.tile([128, NCH * 2 * H], FP16, tag="cc")
            dall = smp.tile([128, NCH * 2 * H], FP16, tag="dall")
            # cc carries the ONES_SCALE factor via exp's bias: the whole A
            # tile (num+den columns) is scaled by OS, so out = v_num * rec
            # with rec = 1/v_den needs no extra constant.
            for br, sc in ((0, 1.0), (1, NEG_SLOPE)):
                nc.scalar.activation(
                    cc[:].rearrange("p (c h two) -> p c h two", c=NCH, two=2)[:, :, :, br],
                    s_sb[:].rearrange("p (c k) -> p c k", c=NCH)[:, :, 0:H],
                    ACTF.Exp, scale=sc, bias=lnos[:])
                nc.scalar.activation(
                    dall[:].rearrange("p (c h two) -> p c h two", c=NCH, two=2)[:, :, :, br],
                    s_sb[:].rearrange("p (c k) -> p c k", c=NCH)[:, :, H:2 * H],
                    ACTF.Exp, scale=sc)

            # s_dst rows: per-chunk PE transpose -> srows [H, N] -> DRAM ->
            # stride-0 broadcast read into all 128 partitions
            srows = smp.tile([H, N], FP16, tag="srows")
            for half in range(2):
                sT_ps = ps_t.tile([H, 256], FP16, tag="t", name="sT_ps")
                for q in range(2):
                    cn = half * 2 + q
                    nc.tensor.transpose(
                        sT_ps[0:H, q * 128:(q + 1) * 128],
                        s_sb[:, cn * 8 + H:cn * 8 + 2 * H], eye_sb[:])
                nc.scalar.copy(srows[:, half * 256:(half + 1) * 256], sT_ps[:])
            nc.sync.dma_start(srt_d[b], srows[:])
            sjb = sjp.tile([128, H * N], FP16, tag="sjb")
            nc.sync.dma_start(
                sjb[:],
                srt_d[b].rearrange("h n -> (h n)").unsqueeze(0)
                    .broadcast_to([128, H * N]))

            # A tiles (Pool): A[:, h*2*AW + br*AW + c] = whaug * CC, then the
            # merged colsums S = [S1|S2] via ones-row matmuls
            s2_ps = ps_t.tile([1, H * 2 * AW], FP32, tag="t", name="s2_ps")
            for cn in range(NCH):
                at, whaug = a_tiles[cn]
                for br in range(2):
                    nc.gpsimd.tensor_tensor(
                        at[:].rearrange("p (h two c) -> p h two c",
                                        h=H, two=2)[:, :, br],
                        whaug[:].rearrange("p (h c) -> p h c", h=H),
                        cc[:, cn * 8:(cn + 1) * 8]
                            .rearrange("p (h two) -> p h two", h=H)[:, :, br]
                            .unsqueeze(2).broadcast_to([128, H, AW]),
                        ALU.mult)
                nc.tensor.matmul(
                    s2_ps[:], ones_col[:], a_tiles[cn][0][:],
                    start=(cn == 0), stop=(cn == NCH - 1))

            # refresh the -S2 row of this batch's iextx tile: build [0 | -S2]
            # on partition 0, then DMA it into partition 66 (compute engines
            # cannot address a write starting at partition 66; DMA can)
            s2n = smp.tile([1, H * 2 * AW], FP16, tag="s2n")
            nc.gpsimd.memset(s2n[:], 0.0)
            nc.vector.tensor_scalar(
                s2n[:].rearrange("p (h two c) -> p h two c", h=H, two=2)[:, :, 1],
                s2_ps[0:1, :]
                    .rearrange("p (h two c) -> p h two c", h=H, two=2)[:, :, 1],
                -1.0, None, ALU.mult)
            iex = iextx[b % 2]
            nc.sync.dma_start(iex[2 * AW:2 * AW + 1, :], s2n[:])

            # ---- per-head: chi + aggregation ----
            pt16_tiles = []
            for h in range(H):
                pt_ps = ps_pt.tile([2 * AW, N], FP32, tag="pt")
                for ic in range(NCH):
                    chi = chp.tile([128, N], FP16, tag="chi")
                    nc.vector.tensor_scalar(
                        chi[:], sjb[:, h * N:(h + 1) * N],
                        ssrcn[:, ic * H + h:ic * H + h + 1],
                        None, ALU.is_ge)
                    at = a_tiles[ic][0]
                    nc.tensor.matmul(
                        pt_ps[:], at[:, h * 2 * AW:(h + 1) * 2 * AW],
                        chi[:], start=(ic == 0), stop=(ic == NCH - 1))

                pt16 = ptp.tile([2 * AW + 1, N], FP16, tag="pt16")
                # ones row lives at partition 66; engine writes must start at a
                # multiple of 32, so memset 64:67 first and let the PT copy
                # overwrite rows 64-65
                nc.gpsimd.memset(pt16[64:2 * AW + 1, :], 1.0)
                nc.scalar.copy(pt16[0:2 * AW, :], pt_ps[:])
                pt16_tiles.append(pt16)

            # ---- per j-chunk tail ----
            outsb = odp.tile([128, NCH * OW], FP32, tag="outsb")
            for jc in range(NCH):
                t_ps = ps_t.tile([128, H * 2 * AW], FP32, tag="t", name="t_ps")
                for h in range(H):
                    nc.tensor.matmul(
                        t_ps[:, h * 2 * AW:(h + 1) * 2 * AW],
                        pt16_tiles[h][:, jc * 128:(jc + 1) * 128],
                        iex[:, h * 2 * AW:(h + 1) * 2 * AW],
                        start=True, stop=True)
                u = tlp.tile([128, H * 2 * AW], FP16, tag="u")
                nc.vector.tensor_tensor(
                    u[:].rearrange("p (ht c) -> p ht c", c=AW),
                    t_ps[:].rearrange("p (ht c) -> p ht c", c=AW),
                    dall[:, jc * 8:(jc + 1) * 8]
                        .unsqueeze(2).broadcast_to([128, 2 * H, AW]),
                    ALU.mult)
                v = tlp.tile([128, H * AW], FP16, tag="v")
                nc.gpsimd.tensor_tensor(
                    v[:].rearrange("p (h c) -> p h c", h=H),
                    u[:].rearrange("p (h two c) -> p h two c", h=H, two=2)[:, :, 0],
                    u[:].rearrange("p (h two c) -> p h two c", h=H, two=2)[:, :, 1],
                    ALU.subtract)
                rec = tlp.tile([128, H], FP32, tag="rec")
                nc.vector.reciprocal(
                    rec[:], v[:].rearrange("p (h c) -> p h c", h=H)[:, :, Dh:Dh + 1]
                        .squeeze(2))
                nc.gpsimd.tensor_tensor(
                    outsb[:, jc * OW:jc * OW + D]
                        .rearrange("p (h c) -> p h c", h=H),
                    v[:].rearrange("p (h c) -> p h c", h=H)[:, :, 0:Dh],
                    rec[:].unsqueeze(2).broadcast_to([128, H, Dh]),
                    ALU.mult)
                nc.vector.tensor_scalar(
                    outsb[:, jc * OW + D:jc * OW + OW],
                    v[:].rearrange("p (h c) -> p h c", h=H)[:, :, Dh:Dh + 1].squeeze(2),
                    1.0 / ONES_SCALE, None, ALU.mult)
            nc.sync.dma_start(
                outden_d[b].rearrange("(p jc) c -> p jc c", p=128),
                outsb[:].rearrange("p (jc c) -> p jc c", jc=NCH))

    nc.compile()
    return nc


def _host_pack(W, a):
    """wt16 = [W | ws] fp16 with ws[:, h] = W_h @ a_src_h, ws[:, H+h] = W_h @ a_dst_h."""
    a_src, a_dst = a[:, :Dh], a[:, Dh:]
    ws = np.zeros((D, 2 * H), dtype=np.float32)
    for h in range(H):
        ws[:, h] = W[:, h * Dh:(h + 1) * Dh] @ a_src[h]
        ws[:, H + h] = W[:, h * Dh:(h + 1) * Dh] @ a_dst[h]
    wt16 = np.concatenate([W, ws], axis=1).astype(np.float16)
    eye = np.eye(D, dtype=np.float16)
    idext = np.zeros((2 * AW, H * 2 * AW), dtype=np.float16)
    i66 = np.eye(2 * AW, dtype=np.float16)
    for h in range(H):
        idext[:2 * AW, h * 2 * AW:(h + 1) * 2 * AW] = i66
    return wt16, eye, idext


def _host_fixup(out, den, h, adj, W, a):
    """Exact correction for masked (adj<=eps) entries, which the device
    ignores.  out'[b,j] = (out*den - P*Whrow) / (den - P) per affected head.
    """
    zer = np.argwhere(adj <= MASK_EPS)
    if zer.shape[0] == 0:
        return out
    a_src, a_dst = a[:, :Dh], a[:, Dh:]
    out = out.copy()
    W64 = W.astype(np.float64)
    wsrc = np.stack([W64[:, hh * Dh:(hh + 1) * Dh] @ a_src[hh].astype(np.float64)
                     for hh in range(H)], axis=1)      # [D, H]
    wdst = np.stack([W64[:, hh * Dh:(hh + 1) * Dh] @ a_dst[hh].astype(np.float64)
                     for hh in range(H)], axis=1)      # [D, H]
    from collections import defaultdict
    cols = defaultdict(list)
    for bb, ii, jj in zer:
        cols[(int(bb), int(jj))].append(int(ii))
    for (bb, jj), iis in cols.items():
        numc = out[bb, jj, :].astype(np.float64) * np.repeat(
            den[bb, jj, :].astype(np.float64), Dh)
        denc = den[bb, jj, :].astype(np.float64).copy()
        s_j = h[bb, jj].astype(np.float64) @ wdst          # [H]
        for ii in iis:
            hi = h[bb, ii].astype(np.float64)
            s_i = hi @ wsrc                                 # [H]
            e = s_i + s_j
            P = np.exp(np.where(e >= 0, e, NEG_SLOPE * e))  # [H]
            Whi = hi @ W64                                  # [D]
            numc -= np.repeat(P, Dh) * Whi
            denc -= P
        if np.any(denc <= 1e-30):
            # fully-masked column: softmax over all-NEG_INF degenerates to
            # uniform over all i (practically unreachable for these inputs).
            Whb = h[bb].astype(np.float64) @ W64
            out[bb, jj, :] = Whb.mean(axis=0).astype(np.float32)
            continue
        out[bb, jj, :] = (numc / np.repeat(denc, Dh)).astype(np.float32)
    return out


def kernel(h, adj, W, a, _trace=False):
    global last_results
    h = np.ascontiguousarray(h, dtype=np.float32)
    adj = np.ascontiguousarray(adj, dtype=np.float32)
    W = np.ascontiguousarray(W, dtype=np.float32)
    a = np.ascontiguousarray(a, dtype=np.float32)

    wt16, eye, idext = _host_pack(W, a)
    h16 = h.astype(np.float16)
    if "nc" not in _CACHE:
        _CACHE["nc"] = _build_nc()
    nc = _CACHE["nc"]

    in_maps = []
    for c in range(NCORES):
        in_maps.append({
            "hx": np.ascontiguousarray(h16[c * NB:(c + 1) * NB]),
            "wt": wt16, "eye": eye, "idext": idext,
        })
    res = bass_utils.run_bass_kernel_spmd(
        nc, in_maps, core_ids=list(range(NCORES)), trace=_trace)
    last_results = res

    outden = np.concatenate([r["outden"] for r in res.results], axis=0)
    out = np.ascontiguousarray(outden[:, :, :D])
    den = np.ascontiguousarray(outden[:, :, D:])
    out = _host_fixup(out, den, h, adj, W, a)
    return out


# revision 17
# speedup vs baseline: 1.3037x; 1.3037x over previous
"""DirectedGATLayer Trainium2 Bass kernel (V3, software-pipelined).

Math: out[b,j,h,:] = sum_i alpha[b,i,j,h] * Wh[b,i,h,:],
alpha = softmax_i( mask(adj) . LeakyReLU_0.2(s_src[b,i,h] + s_dst[b,j,h]) )

Key identity: exp(leaky(e)) = chi*exp(e) + (1-chi)*exp(0.2e) with
chi = [e >= 0], and exp(e) = exp(s_i)exp(s_j) is rank-1.  Only the
binary chi is materialized at NxN size (one DVE tensor-scalar compare
per tile); all exponentials live on N-sized vectors, and the two leaky
branches plus softmax denominators are accumulated by TensorE matmuls:

  PT[c,j]   = sum_i A[i,c] * chi[i,j],   A = [C1*Whaug | C2*Whaug]
  num/den   = D1_j*PT1 + D2_j*(S2 - PT2)

A columns are laid out (head, c, branch) with the branch pair innermost
so the A-build multiply runs in the DVE 16-bit fast mode.  The -S2
correction rides as an extra row of the (otherwise constant) identity
operand of the per-j-chunk transpose matmul, paired with an extra
ones-row in pt16 — no seed matmuls needed.  The s_dst row broadcast to
all 128 partitions goes through a DRAM round trip (SBUF -> DRAM ->
stride-0 broadcast read): the DMA engines are otherwise idle, and a
single-partition SBUF source caps at the per-partition port rate.

Batches are software-pipelined: phase A of batch b+1 (load, transpose,
Wh, s-vectors, broadcast round trip, A-tiles) is emitted between the
chi/PT heads and the per-j-chunk tail of batch b, so the long DMA
latency chain hides behind compute and every engine queue stays fed.

adj enters only through exact-zero entries (uniform[0,1) inputs); those
(~1 per run) are corrected exactly on the host from the returned den.

Data parallel over 8 cores: 4 batches per core, identical NEFF (SPMD).
"""
import sys

if '/opt/trn_rl_repo' not in sys.path:
    sys.path.insert(0, '/opt/trn_rl_repo')

import numpy as np
from contextlib import ExitStack

import concourse.bass as bass
import concourse.tile as tile
from concourse import bacc, mybir
from concourse import bass_utils

FP32 = mybir.dt.float32
FP16 = mybir.dt.float16
ALU = mybir.AluOpType
ACTF = mybir.ActivationFunctionType

B, N, D, H, Dh = 32, 512, 128, 4, 32
NB = 4            # batches per core
NCORES = 8
NCH = N // 128    # 4 chunks of 128 along n
AW = 33           # A-block width per head per branch (32 Wh cols + ones col)
OW = D + H        # out+den column block per j-chunk (132)
ONES_SCALE = 1.0 / 64.0   # keep fp16 denominators in range (folded into cc)
NEG_SLOPE = 0.2
MASK_EPS = 1e-8

_CACHE = {}
last_results = None   # BassKernelResults of the most recent run (for test.py)


def _build_nc():
    nc = bacc.Bacc("TRN2", target_bir_lowering=False, debug=False,
                   num_devices=NCORES)

    hx_d = nc.dram_tensor("hx", [NB, N, D], FP16, kind="ExternalInput").ap()
    wt_d = nc.dram_tensor("wt", [D, D + 2 * H], FP16, kind="ExternalInput").ap()
    eye_d = nc.dram_tensor("eye", [D, D], FP16, kind="ExternalInput").ap()
    idext_d = nc.dram_tensor("idext", [2 * AW, H * 2 * AW], FP16,
                             kind="ExternalInput").ap()
    srt_d = nc.dram_tensor("srt", [NB, H, N], FP16, kind="Internal").ap()
    outden_d = nc.dram_tensor("outden", [NB, N, OW], FP32,
                              kind="ExternalOutput").ap()

    with tile.TileContext(nc) as tc, ExitStack() as ctx:
        cpool = ctx.enter_context(tc.tile_pool(name="const", bufs=1))
        sbp = ctx.enter_context(tc.tile_pool(name="sb", bufs=2))
        whp = ctx.enter_context(tc.tile_pool(name="wha", bufs=6))
        ap_pool = ctx.enter_context(tc.tile_pool(name="atile", bufs=8))
        chp = ctx.enter_context(tc.tile_pool(name="chi", bufs=6))
        ptp = ctx.enter_context(tc.tile_pool(name="pt16", bufs=5))
        sjp = ctx.enter_context(tc.tile_pool(name="sjb", bufs=3))
        smp = ctx.enter_context(tc.tile_pool(name="small", bufs=3))
        tlp = ctx.enter_context(tc.tile_pool(name="tail", bufs=2))
        odp = ctx.enter_context(tc.tile_pool(name="od", bufs=2))

        ps_h = ctx.enter_context(tc.tile_pool(name="psH", bufs=1, space="PSUM"))
        ps_wh = ctx.enter_context(tc.tile_pool(name="psW", bufs=2, space="PSUM"))
        ps_pt = ctx.enter_context(tc.tile_pool(name="psPT", bufs=2, space="PSUM"))
        ps_t = ctx.enter_context(tc.tile_pool(name="psT", bufs=3, space="PSUM"))

        # constants
        wt_sb = cpool.tile([D, D + 2 * H], FP16, tag="wt")
        nc.gpsimd.dma_start(wt_sb[:], wt_d[:])
        eye_sb = cpool.tile([D, D], FP16, tag="eye")
        nc.gpsimd.dma_start(eye_sb[:], eye_d[:])
        ones_col = cpool.tile([D, 1], FP16, tag="ones")
        nc.gpsimd.memset(ones_col[:], 1.0)
        lnos = cpool.tile([128, 1], FP32, tag="lnos")
        nc.gpsimd.memset(lnos[:], float(np.log(ONES_SCALE)))
        iextx = [cpool.tile([2 * AW + 1, H * 2 * AW], FP16, tag=f"iextx{k}",
                            name=f"iextx{k}")
                 for k in range(2)]
        for k in range(2):
            nc.gpsimd.dma_start(iextx[k][0:2 * AW, :], idext_d[:])
        actwarm = cpool.tile([1, 1], FP32, tag="actwarm")
        nc.scalar.activation(actwarm[:], ones_col[0:1, 0:1], ACTF.Exp)

        st = [dict() for _ in range(NB)]   # per-batch pipeline state

        def s_load(b):
            hsb = sbp.tile([128, N], FP16, tag="hsb", name="hsb")
            nc.sync.dma_start(
                hsb[:].rearrange("p (c d) -> p c d", c=NCH),
                hx_d[b].rearrange("(p c) d -> p c d", p=128))
            st[b]["hsb"] = hsb

        def s_trans(b):
            hsb = st[b]["hsb"]
            hT_ps = ps_h.tile([128, N], FP16, tag="hT", name="hT_ps")
            for ic in range(NCH):
                nc.tensor.transpose(hT_ps[:, ic * 128:(ic + 1) * 128],
                                    hsb[:, ic * 128:(ic + 1) * 128], eye_sb[:])
            hT = sbp.tile([128, N], FP16, tag="hT16", name="hT")
            nc.scalar.copy(hT[:], hT_ps[:])
            st[b]["hT"] = hT

        def s_wh(b):
            hT = st[b]["hT"]
            # s_sb holds only s_dst; ssrcn = -s_src feeds both the compares
            # and (via a negative activation scale) the cc exponentials
            s_sb = smp.tile([128, NCH * H], FP16, tag="s_sb", name="s_sb")
            ssrcn = smp.tile([128, NCH * H], FP32, tag="ssrcn", name="ssrcn")
            a_tiles = []
            for cn in range(NCH):
                wh_ps = ps_wh.tile([128, D + 2 * H], FP32, tag="wh",
                                   name="wh_ps")
                nc.tensor.matmul(wh_ps[:], hT[:, cn * 128:(cn + 1) * 128],
                                 wt_sb[:], start=True, stop=True)
                # whaug2: [128, (h, c, two)] with Wh duplicated on the branch
                # pair and 1.0 in the c=32 ones slots
                whaug = whp.tile([128, H * 2 * AW], FP16, tag="whaug",
                                 name="whaug")
                nc.gpsimd.memset(
                    whaug[:].rearrange("p (h c two) -> p h c two",
                                       h=H, two=2)[:, :, Dh:Dh + 1],
                    1.0)
                nc.scalar.copy(
                    whaug[:].rearrange("p (h c two) -> p h c two",
                                       h=H, two=2)[:, :, 0:Dh],
                    wh_ps[:, 0:D].rearrange("p (h c) -> p h c", h=H)
                        .unsqueeze(3).broadcast_to([128, H, Dh, 2]))
                nc.vector.tensor_scalar(
                    ssrcn[:, cn * H:(cn + 1) * H], wh_ps[:, D:D + H],
                    -1.0, None, ALU.mult)
                nc.scalar.copy(s_sb[:, cn * H:(cn + 1) * H],
                               wh_ps[:, D + H:D + 2 * H])
                at = ap_pool.tile([128, H * 2 * AW], FP16, tag="A", name="at")
                a_tiles.append((at, whaug))
            cc = smp.tile([128, NCH * 2 * H], FP16, tag="cc", name="cc")
            dall = smp.tile([128, NCH * 2 * H], FP16, tag="dall", name="dall")
            # cc = OS * exp(sc * s_src) via exp(-sc * ssrcn + ln OS)
            for br, sc in ((0, 1.0), (1, NEG_SLOPE)):
                nc.scalar.activation(
                    cc[:].rearrange("p (c h two) -> p c h two",
                                    c=NCH, two=2)[:, :, :, br],
                    ssrcn[:].rearrange("p (c h) -> p c h", c=NCH),
                    ACTF.Exp, scale=-sc, bias=lnos[:])
                nc.scalar.activation(
                    dall[:].rearrange("p (c h two) -> p c h two",
                                      c=NCH, two=2)[:, :, :, br],
                    s_sb[:].rearrange("p (c h) -> p c h", c=NCH),
                    ACTF.Exp, scale=sc)
            st[b].update(s_sb=s_sb, ssrcn=ssrcn, cc=cc, dall=dall,
                         a_tiles=a_tiles)

        def s_srow(b):
            s_sb = st[b]["s_sb"]
            srows = smp.tile([H, N], FP16, tag="srows", name="srows")
            for half in range(2):
                sT_ps = ps_t.tile([H, 256], FP16, tag="t", name="sT_ps")
                for q in range(2):
                    cn = half * 2 + q
                    nc.tensor.transpose(
                        sT_ps[0:H, q * 128:(q + 1) * 128],
                        s_sb[:, cn * H:(cn + 1) * H], eye_sb[:])
                nc.scalar.copy(srows[:, half * 256:(half + 1) * 256], sT_ps[:])
            nc.sync.dma_start(srt_d[b], srows[:])
            sjb = sjp.tile([128, H * N], FP16, tag="sjb", name="sjb")
            nc.sync.dma_start(
                sjb[:],
                srt_d[b].rearrange("h n -> (h n)").unsqueeze(0)
                    .broadcast_to([128, H * N]))
            st[b]["sjb"] = sjb

        def s_abuild(b):
            cc = st[b]["cc"]
            a_tiles = st[b]["a_tiles"]
            s2_ps = ps_t.tile([1, H * 2 * AW], FP32, tag="t", name="s2_ps")
            for cn in range(NCH):
                at, whaug = a_tiles[cn]
                # branch pair innermost (stride 1, count 2) -> DVE 16-bit mode
                nc.vector.tensor_tensor(
                    at[:].rearrange("p (h c two) -> p h c two", h=H, two=2),
                    whaug[:].rearrange("p (h c two) -> p h c two", h=H, two=2),
                    cc[:, cn * 8:(cn + 1) * 8]
                        .rearrange("p (h two) -> p h two", h=H)
                        .unsqueeze(2).broadcast_to([128, H, AW, 2]),
                    ALU.mult)
                nc.tensor.matmul(
                    s2_ps[:], ones_col[:], at[:],
                    start=(cn == 0), stop=(cn == NCH - 1))
            # [0 | -S2] into partition 66 of this batch's iextx (via DMA:
            # compute engines cannot start a write at partition 66)
            s2n = smp.tile([1, H * 2 * AW], FP16, tag="s2n", name="s2n")
            nc.gpsimd.memset(s2n[:], 0.0)
            nc.vector.tensor_scalar(
                s2n[:].rearrange("p (h c two) -> p h c two", h=H, two=2)[:, :, :, 1],
                s2_ps[0:1, :].rearrange("p (h c two) -> p h c two",
                                        h=H, two=2)[:, :, :, 1],
                -1.0, None, ALU.mult)
            iex = iextx[b % 2]
            nc.sync.dma_start(iex[2 * AW:2 * AW + 1, :], s2n[:])
            st[b]["iex"] = iex

        def s_head(b, h):
            sjb, ssrcn = st[b]["sjb"], st[b]["ssrcn"]
            a_tiles = st[b]["a_tiles"]
            pt_ps = ps_pt.tile([2 * AW, N], FP32, tag="pt", name="pt_ps")
            for ic in range(NCH):
                chi = chp.tile([128, N], FP16, tag="chi", name="chi")
                nc.vector.tensor_scalar(
                    chi[:], sjb[:, h * N:(h + 1) * N],
                    ssrcn[:, ic * H + h:ic * H + h + 1],
                    None, ALU.is_ge)
                at = a_tiles[ic][0]
                nc.tensor.matmul(
                    pt_ps[:], at[:, h * 2 * AW:(h + 1) * 2 * AW],
                    chi[:], start=(ic == 0), stop=(ic == NCH - 1))
            pt16 = ptp.tile([2 * AW + 1, N], FP16, tag="pt16", name="pt16")
            # ones row lives at partition 66; engine writes must start at a
            # multiple of 32, so memset 64:67 and let the copy redo 64-65
            nc.gpsimd.memset(pt16[64:2 * AW + 1, :], 1.0)
            nc.scalar.copy(pt16[0:2 * AW, :], pt_ps[:])
            st[b].setdefault("pt16", []).append(pt16)

        def s_tail(b, jc):
            dall, iex = st[b]["dall"], st[b]["iex"]
            pt16_tiles = st[b]["pt16"]
            outsb = st[b]["outsb"]
            t_ps = ps_t.tile([128, H * 2 * AW], FP32, tag="t", name="t_ps")
            for h in range(H):
                nc.tensor.matmul(
                    t_ps[:, h * 2 * AW:(h + 1) * 2 * AW],
                    pt16_tiles[h][:, jc * 128:(jc + 1) * 128],
                    iex[:, h * 2 * AW:(h + 1) * 2 * AW],
                    start=True, stop=True)
            u = tlp.tile([128, H * 2 * AW], FP16, tag="u", name="u")
            nc.vector.tensor_tensor(
                u[:].rearrange("p (h c two) -> p h c two", h=H, two=2),
                t_ps[:].rearrange("p (h c two) -> p h c two", h=H, two=2),
                dall[:, jc * 8:(jc + 1) * 8]
                    .rearrange("p (h two) -> p h two", h=H)
                    .unsqueeze(2).broadcast_to([128, H, AW, 2]),
                ALU.mult)
            v = tlp.tile([128, H * AW], FP16, tag="v", name="v")
            nc.gpsimd.tensor_tensor(
                v[:].rearrange("p (h c) -> p h c", h=H),
                u[:].rearrange("p (h c two) -> p h c two", h=H, two=2)[:, :, :, 0],
                u[:].rearrange("p (h c two) -> p h c two", h=H, two=2)[:, :, :, 1],
                ALU.subtract)
            rec = tlp.tile([128, H], FP32, tag="rec", name="rec")
            nc.vector.reciprocal(
                rec[:], v[:].rearrange("p (h c) -> p h c", h=H)[:, :, Dh:Dh + 1]
                    .squeeze(2))
            nc.gpsimd.tensor_tensor(
                outsb[:, jc * OW:jc * OW + D]
                    .rearrange("p (h c) -> p h c", h=H),
                v[:].rearrange("p (h c) -> p h c", h=H)[:, :, 0:Dh],
                rec[:].unsqueeze(2).broadcast_to([128, H, Dh]),
                ALU.mult)
            nc.scalar.activation(
                outsb[:, jc * OW + D:jc * OW + OW],
                v[:].rearrange("p (h c) -> p h c", h=H)[:, :, Dh:Dh + 1].squeeze(2),
                ACTF.Copy, scale=1.0 / ONES_SCALE)

        def s_store(b):
            nc.sync.dma_start(
                outden_d[b].rearrange("(p jc) c -> p jc c", p=128),
                st[b]["outsb"][:].rearrange("p (jc c) -> p jc c", jc=NCH))

        # ---- skewed pipeline: phase A of b+1 hides inside phase B of b ----
        s_load(0)
        s_trans(0)
        s_wh(0)
        s_srow(0)
        s_abuild(0)
        if NB > 1:
            s_load(1)
        for b in range(NB):
            st[b]["outsb"] = odp.tile([128, NCH * OW], FP32, tag="outsb",
                                      name="outsb")
            s_head(b, 0)
            s_head(b, 1)
            if b + 1 < NB:
                s_trans(b + 1)
                s_wh(b + 1)
            s_head(b, 2)
            s_head(b, 3)
            if b + 1 < NB:
                s_srow(b + 1)
                s_abuild(b + 1)
            if b + 2 < NB:
                s_load(b + 2)
            for jc in range(NCH):
                s_tail(b, jc)
            s_store(b)

    nc.compile()
    return nc


def _host_pack(W, a):
    """wt16 = [W | ws] fp16 with ws[:, h] = W_h @ a_src_h, ws[:, H+h] = W_h @ a_dst_h."""
    a_src, a_dst = a[:, :Dh], a[:, Dh:]
    ws = np.zeros((D, 2 * H), dtype=np.float32)
    for h in range(H):
        ws[:, h] = W[:, h * Dh:(h + 1) * Dh] @ a_src[h]
        ws[:, H + h] = W[:, h * Dh:(h + 1) * Dh] @ a_dst[h]
    wt16 = np.concatenate([W, ws], axis=1).astype(np.float16)
    eye = np.eye(D, dtype=np.float16)
    idext = np.zeros((2 * AW, H * 2 * AW), dtype=np.float16)
    i66 = np.eye(2 * AW, dtype=np.float16)
    for h in range(H):
        idext[:2 * AW, h * 2 * AW:(h + 1) * 2 * AW] = i66
    return wt16, eye, idext


def _host_fixup(out, den, h, adj, W, a):
    """Exact correction for masked (adj<=eps) entries, which the device
    ignores.  out'[b,j] = (out*den - P*Whrow) / (den - P) per affected head.
    """
    zer = np.argwhere(adj <= MASK_EPS)
    if zer.shape[0] == 0:
        return out
    a_src, a_dst = a[:, :Dh], a[:, Dh:]
    out = out.copy()
    W64 = W.astype(np.float64)
    wsrc = np.stack([W64[:, hh * Dh:(hh + 1) * Dh] @ a_src[hh].astype(np.float64)
                     for hh in range(H)], axis=1)      # [D, H]
    wdst = np.stack([W64[:, hh * Dh:(hh + 1) * Dh] @ a_dst[hh].astype(np.float64)
                     for hh in range(H)], axis=1)      # [D, H]
    from collections import defaultdict
    cols = defaultdict(list)
    for bb, ii, jj in zer:
        cols[(int(bb), int(jj))].append(int(ii))
    for (bb, jj), iis in cols.items():
        numc = out[bb, jj, :].astype(np.float64) * np.repeat(
            den[bb, jj, :].astype(np.float64), Dh)
        denc = den[bb, jj, :].astype(np.float64).copy()
        s_j = h[bb, jj].astype(np.float64) @ wdst          # [H]
        for ii in iis:
            hi = h[bb, ii].astype(np.float64)
            s_i = hi @ wsrc                                 # [H]
            e = s_i + s_j
            P = np.exp(np.where(e >= 0, e, NEG_SLOPE * e))  # [H]
            Whi = hi @ W64                                  # [D]
            numc -= np.repeat(P, Dh) * Whi
            denc -= P
        if np.any(denc <= 1e-30):
            # fully-masked column: softmax over all-NEG_INF degenerates to
            # uniform over all i (practically unreachable for these inputs).
            Whb = h[bb].astype(np.float64) @ W64
            out[bb, jj, :] = Whb.mean(axis=0).astype(np.float32)
            continue
        out[bb, jj, :] = (numc / np.repeat(denc, Dh)).astype(np.float32)
    return out


def kernel(h, adj, W, a, _trace=False):
    global last_results
    h = np.ascontiguousarray(h, dtype=np.float32)
    adj = np.ascontiguousarray(adj, dtype=np.float32)
    W = np.ascontiguousarray(W, dtype=np.float32)
    a = np.ascontiguousarray(a, dtype=np.float32)

    wt16, eye, idext = _host_pack(W, a)
    h16 = h.astype(np.float16)
    if "nc" not in _CACHE:
        _CACHE["nc"] = _build_nc()
    nc = _CACHE["nc"]

    in_maps = []
    for c in range(NCORES):
        in_maps.append({
            "hx": np.ascontiguousarray(h16[c * NB:(c + 1) * NB]),
            "wt": wt16, "eye": eye, "idext": idext,
        })
    res = bass_utils.run_bass_kernel_spmd(
        nc, in_maps, core_ids=list(range(NCORES)), trace=_trace)
    last_results = res

    outden = np.concatenate([r["outden"] for r in res.results], axis=0)
    out = np.ascontiguousarray(outden[:, :, :D])
    den = np.ascontiguousarray(outden[:, :, D:])
    out = _host_fixup(out, den, h, adj, W, a)
    return out
